# revision 5
# baseline (speedup 1.0000x reference)
"""Trainium2 Bass kernel: masked-mean-pool -> linear projection -> pairwise L2.

Full computation:
    pooled = einsum('nlh,nl->nh', inputs, masks) / sum(masks, 1)   # [N, H]
    emb    = pooled @ W + b                                         # [N, H]
    out    = pairwise_l2(emb)                                       # [N, N]

Sharding: rows (N) split across 8 NeuronCores; each core pools/projects its
512-row shard, all-gathers an augmented embedding payload [-2*embT; sqnorm_row]
([513, 512] f32 per rank), and computes its [512, 4096] block of the distance
matrix with a single augmented matmul:
    psum[i, j] = sum_h embT[h,i] * (-2*embT[h,j]) + 1 * sn[j]  (K = 512 + 1)
    dist[i, j] = sqrt(max(psum[i,j] + sn[i], 0))
Host concatenates the 8 row-blocks and zeroes the diagonal.
"""

import sys
import numpy as np

if "/opt/trn_rl_repo" not in sys.path:
    sys.path.insert(0, "/opt/trn_rl_repo")

N_TOTAL, L, H = 4096, 64, 512
R = 8                    # cores
NS = N_TOTAL // R        # 512 rows per core
NB = NS // 128           # 4 n-blocks of 128 partitions
HT = H // 128            # 4 h-tiles of 128
LC = 4                   # l-chunks per n-block
LCS = L // LC            # 16 l per chunk
AUG = H + 1              # payload rows: 512 emb + 1 sq-norm

_CACHE = {}


def _build_nc(use_masks: bool):
    import concourse.bacc as bacc
    import concourse.tile as tile
    import concourse.mybir as mybir

    f32 = mybir.dt.float32
    ALU = mybir.AluOpType
    ACT = mybir.ActivationFunctionType

    nc = bacc.Bacc(
        "TRN2",
        target_bir_lowering=False,
        debug=False,
        enable_asserts=False,
        num_devices=R,
    )

    x_ext = nc.dram_tensor("inputs", [NS, L, H], f32, kind="ExternalInput")
    if use_masks:
        mw_ext = nc.dram_tensor("mw", [NS, L], f32, kind="ExternalInput")
    w_ext = nc.dram_tensor("W", [H, H], f32, kind="ExternalInput")
    b_ext = nc.dram_tensor("b", [H], f32, kind="ExternalInput")
    out_ext = nc.dram_tensor("out", [NS, N_TOTAL], f32, kind="ExternalOutput")

    ident_dram = nc.inline_tensor(np.eye(128, dtype=np.float32), name="ident")

    with tile.TileContext(nc) as tc:
        with (
            tc.tile_pool(name="const", bufs=1) as cpool,
            tc.tile_pool(name="xp", bufs=3) as xpool,
            tc.tile_pool(name="rp", bufs=2) as rpool,
            tc.tile_pool(name="ep", bufs=4) as epool,
            tc.tile_pool(name="dram", bufs=1, space="DRAM") as dpool,
        ):
            # ---- constants / weights ----
            ident_sb = cpool.tile([128, 128], f32, name="ident_sb")
            nc.sync.dma_start(ident_sb[:, :], ident_dram[:, :])

            w_sb = cpool.tile([128, HT, H], f32, name="w_sb")
            for k in range(HT):
                nc.sync.dma_start(w_sb[:, k, :], w_ext[k * 128:(k + 1) * 128, :])

            b_ap = b_ext.ap().rearrange("(x y) -> x y", y=1)  # [512, 1]
            b_sb = cpool.tile([128, HT], f32, name="b_sb")
            for m in range(HT):
                nc.sync.dma_start(b_sb[:, m:m + 1], b_ap[m * 128:(m + 1) * 128, 0:1])
            b2_sb = cpool.tile([128, HT], f32, name="b2_sb")
            nc.vector.tensor_scalar_mul(b2_sb[:, :], b_sb[:, :], -2.0)

            ones_col = cpool.tile([128, 1], f32, name="ones_col")
            nc.vector.memset(ones_col[:, :], 1.0)
            ones_row = cpool.tile([1, 128], f32, name="ones_row")
            nc.vector.memset(ones_row[:, :], 1.0)

            if use_masks:
                mw_sb = cpool.tile([128, NB, L], f32, name="mw_sb")
                for nb in range(NB):
                    nc.sync.dma_start(
                        mw_sb[:, nb, :], mw_ext[nb * 128:(nb + 1) * 128, :]
                    )

            pooled_sb = cpool.tile([128, NB, H], f32, name="pooled_sb")

            # ---- phase 1: masked-mean pooling (n on partitions) ----
            for nb in range(NB):
                for lc in range(LC):
                    xt = xpool.tile([128, LCS, H], f32, name="xt")
                    nc.sync.dma_start(
                        xt[:, :, :],
                        x_ext[nb * 128:(nb + 1) * 128, lc * LCS:(lc + 1) * LCS, :],
                    )
                    if use_masks:
                        for l in range(LCS):
                            gl = lc * LCS + l
                            nc.scalar.mul(
                                xt[:, l, :], xt[:, l, :], mw_sb[:, nb, gl:gl + 1]
                            )
                    # in-place binary-tree sum over the l axis: 16 -> 8 -> ... -> 1
                    half = LCS
                    while half > 1:
                        half //= 2
                        nc.vector.tensor_add(
                            xt[:, 0:half, :], xt[:, 0:half, :], xt[:, half:2 * half, :]
                        )
                    if lc == 0:
                        nc.vector.tensor_copy(pooled_sb[:, nb, :], xt[:, 0, :])
                    else:
                        nc.vector.tensor_add(
                            pooled_sb[:, nb, :], pooled_sb[:, nb, :], xt[:, 0, :]
                        )

            # ---- phase 2a: transpose pooled -> pooledT (h on partitions) ----
            pooledT_sb = cpool.tile([128, HT, NS], f32, name="pooledT_sb")
            with tc.tile_pool(name="pst", bufs=2, space="PSUM") as tpool:
                for ht in range(HT):
                    for nb in range(NB):
                        pst = tpool.tile([128, 128], f32, name="pst")
                        nc.tensor.transpose(
                            pst[:, :],
                            pooled_sb[:, nb, ht * 128:(ht + 1) * 128],
                            ident_sb[:, :],
                        )
                        nc.vector.tensor_copy(
                            pooledT_sb[:, ht, nb * 128:(nb + 1) * 128], pst[:, :]
                        )

            # ---- phase 2b: projection embT = W.T-contract(pooledT) + b ----
            embT_sb = cpool.tile([128, HT, NS], f32, name="embT_sb")
            scaledT_sb = cpool.tile([128, HT, NS], f32, name="scaledT_sb")
            with tc.tile_pool(name="psp", bufs=2, space="PSUM") as ppool:
                for m in range(HT):
                    psp = ppool.tile([128, NS], f32, name="psp")
                    for k in range(HT):
                        nc.tensor.matmul(
                            psp[:, :],
                            w_sb[:, k, m * 128:(m + 1) * 128],
                            pooledT_sb[:, k, :],
                            start=(k == 0),
                            stop=(k == HT - 1),
                        )
                    nc.scalar.activation(
                        embT_sb[:, m, :], psp[:, :], ACT.Identity,
                        bias=b_sb[:, m:m + 1], scale=1.0,
                    )
                    nc.scalar.activation(
                        scaledT_sb[:, m, :], psp[:, :], ACT.Identity,
                        bias=b2_sb[:, m:m + 1], scale=-2.0,
                    )

            # ---- phase 2c: squared norms ----
            sq_sb = cpool.tile([128, HT, NS], f32, name="sq_sb")
            for k in range(HT):
                nc.scalar.square(sq_sb[:, k, :], embT_sb[:, k, :])

            sn_row_sb = cpool.tile([1, NS], f32, name="sn_row_sb")
            sn_col_sb = cpool.tile([128, HT], f32, name="sn_col_sb")
            with tc.tile_pool(name="psn", bufs=1, space="PSUM") as npool:
                ps_snrow = npool.tile([1, NS], f32, name="ps_snrow")
                for k in range(HT):
                    nc.tensor.matmul(
                        ps_snrow[0:1, :], ones_col[:, 0:1], sq_sb[:, k, :],
                        start=(k == 0), stop=(k == HT - 1),
                    )
                nc.scalar.copy(sn_row_sb[0:1, :], ps_snrow[0:1, :])

                for m in range(HT):
                    ps_sncol = npool.tile([128, 1], f32, name="ps_sncol", bufs=2)
                    for k in range(HT):
                        nc.tensor.matmul(
                            ps_sncol[:, 0:1],
                            sq_sb[:, k, m * 128:(m + 1) * 128],
                            ones_col[:, 0:1],
                            start=(k == 0),
                            stop=(k == HT - 1),
                        )
                    nc.scalar.copy(sn_col_sb[:, m:m + 1], ps_sncol[:, 0:1])

            # ---- phase 2d: all-gather payload [-2*embT ; sn_row] ----
            payload_d = dpool.tile([AUG, NS], f32, name="payload_d")
            gathered_d = dpool.tile(
                [R * AUG, NS], f32, name="gathered_d", addr_space="Shared"
            )
            for k in range(HT):
                nc.sync.dma_start(
                    payload_d[k * 128:(k + 1) * 128, :], scaledT_sb[:, k, :]
                )
            nc.sync.dma_start(payload_d[H:H + 1, :], sn_row_sb[0:1, :])
            nc.gpsimd.collective_compute(
                "AllGather",
                ALU.bypass,
                replica_groups=[list(range(R))],
                ins=[payload_d.opt()],
                outs=[gathered_d.opt()],
            )

            # ---- phase 3: distance blocks ----
            bpool_cm = tc.tile_pool(name="psb", bufs=4, space="PSUM")
            bpool = bpool_cm.__enter__()
            for jb in range(R):
                rhst = rpool.tile([128, HT, NS], f32, name="rhst")
                snr = rpool.tile([1, NS], f32, name="snr")
                base = jb * AUG
                for k in range(HT):
                    nc.sync.dma_start(
                        rhst[:, k, :],
                        gathered_d[base + k * 128:base + (k + 1) * 128, :],
                    )
                nc.sync.dma_start(snr[0:1, :], gathered_d[base + H:base + H + 1, :])
                for m in range(HT):
                    ps = bpool.tile([128, NS], f32, name="ps")
                    nc.tensor.matmul(
                        ps[:, :], ones_row[0:1, :], snr[0:1, :],
                        start=True, stop=False,
                    )
                    for k in range(HT):
                        nc.tensor.matmul(
                            ps[:, :],
                            embT_sb[:, k, m * 128:(m + 1) * 128],
                            rhst[:, k, :],
                            start=False,
                            stop=(k == HT - 1),
                        )
                    sqt = epool.tile([128, NS], f32, name="sqt")
                    nc.vector.tensor_scalar(
                        sqt[:, :], ps[:, :], sn_col_sb[:, m:m + 1], 0.0,
                        op0=ALU.add, op1=ALU.max,
                    )
                    nc.scalar.sqrt(sqt[:, :], sqt[:, :])
                    nc.sync.dma_start(
                        out_ext[m * 128:(m + 1) * 128, jb * NS:(jb + 1) * NS],
                        sqt[:, :],
                    )
            bpool_cm.__exit__(None, None, None)

    nc.compile()
    return nc


def _get_nc(use_masks: bool):
    if use_masks not in _CACHE:
        _CACHE[use_masks] = _build_nc(use_masks)
    return _CACHE[use_masks]


def _run_device(x, mw, w_eff, b, trace=False, trace_cores=None):
    from concourse import bass_utils

    use_masks = mw is not None
    nc = _get_nc(use_masks)
    in_maps = []
    for r in range(R):
        m = {
            "inputs": np.ascontiguousarray(x[r * NS:(r + 1) * NS]),
            "W": w_eff,
            "b": b,
        }
        if use_masks:
            m["mw"] = np.ascontiguousarray(mw[r * NS:(r + 1) * NS])
        in_maps.append(m)
    res = bass_utils.run_bass_kernel_spmd(
        nc,
        in_maps,
        core_ids=list(range(R)),
        trace=trace,
        trace_cores=trace_cores,
    )
    out = np.concatenate([res.results[r]["out"] for r in range(R)], axis=0)
    np.fill_diagonal(out, 0.0)
    return out, res


def kernel(inputs, masks, W, b):
    inputs = np.ascontiguousarray(np.asarray(inputs, dtype=np.float32))
    masks = np.asarray(masks, dtype=np.float32)
    W = np.ascontiguousarray(np.asarray(W, dtype=np.float32))
    b = np.ascontiguousarray(np.asarray(b, dtype=np.float32))

    denom = masks.sum(axis=1, keepdims=True)
    row_uniform = bool(np.all(masks == masks[:, :1])) and bool(np.all(denom != 0))
    if row_uniform:
        # uniform per-row masks cancel: pooled = mean over L; fold 1/L into W
        w_eff = np.ascontiguousarray(W / np.float32(L))
        out, _ = _run_device(inputs, None, w_eff, b)
    else:
        mw = np.ascontiguousarray((masks / denom).astype(np.float32))
        out, _ = _run_device(inputs, mw, W, b)
    return out
